# Initial kernel scaffold
#
"""Multi-head attention (B=4, L=1024, D=1024, H=16) on 8 TRN2 NeuronCores.

Sharding: pure data-parallel over (batch, query-half) — core c handles batch
c//2, query rows [512*(c%2), 512*(c%2+1)). No collectives; the host
concatenates the 8 output slices.

v2 rewrite (from trace analysis of the v1 kernel):
  * DMA loads are phase-serialized (qT+Wq -> xT+Wk -> Wv+Wo) via gate ops +
    manual deps so the first projection starts at ~9us instead of ~33us
    (v1 let all 16 DMA rings run concurrently -> everything landed at ~30us).
  * Keep-alive matmuls warm the PE HAM clock-gate during the load window.
  * Attention starts right after Q-proj + K-proj(0) (~25us, v1: ~66us);
    remaining K/V projections are PE-queue filler inside the pair stream.
  * O matmuls are column-tiled (two M=64 heads at tile_position (0,0)/(0,64))
    instead of two serial M=65 — the ones-column denominator is replaced by
    a DVE+GPSIMD exp-sum accumulation and two col-tiled M=1 matmuls.
  * Each pair's scale (sr) matmuls are deferred ~2us of PE work into the next
    pair so the reciprocal chain never blocks the in-order PE queue (v1 lost
    ~4.9us/pair to this), and the reciprocal is the single-pass approx_fast.
  * Scale multiply reads O-PSUM and sr-PSUM directly (no oTu copy).
  * bv bias is folded into the V cast (kills 16 K=1 bias matmuls).

Layouts (all transposed, no transposes anywhere):
  Q^T[vd, q] = Wq(lhsT) @ qT(rhs)  (+bq per-partition)
  K^T[vd, k] = Wk(lhsT) @ xT(rhs)  (+bk per-partition)
  V  [k, vd] = xT(lhsT) @ Wv(rhs)  (+bv via DVE add of bv_rep)
  S^T[k, 2, q] = K^T_h(lhsT, K=64) @ Q^T_h  per head PAIR, row-tiled
  es = exp(S^T/8 + kmask_bias)   (ScalarE, PSUM->SBUF bf16)
  acc = sum_kt es                (DVE half + GPSIMD half, bf16)
  den = ones^T @ acc             (two col-tiled M=1 matmuls)
  O^T[128, q] = [V_h0|V_h1](lhsT, M=64 each, col-tiled) @ es
  oTs = O^T * (1/den broadcast)  (DVE, both operands PSUM)
  out[q, d] = (oTs.T @ Wo) * q_mask + bo  (DVE STT epilogue)
"""

import os

os.environ.setdefault("MYCRO_LOCAL_CACHE", "1")

import numpy as np
import ml_dtypes

BF16 = ml_dtypes.bfloat16

B, LQ, LK = 4, 1024, 1024
D = 1024  # QD = KD = VD
H, DH = 16, 64
QS = 512  # queries per core
NCORES = 8
NEG = -1e4  # additive key-mask bias (exp(-1e4) == 0)

_NC_CACHE = {}


def _build_nc():
    import concourse.bacc as bacc
    import concourse.mybir as mybir
    import concourse.tile as tile

    dt = mybir.dt

    nc = bacc.Bacc(
        "TRN2",
        debug=False,
        target_bir_lowering=False,
        num_devices=NCORES,
    )

    def din(name, shape, dtype):
        return nc.dram_tensor(name, shape, dtype, kind="ExternalInput").ap()

    aps = {
        "qT": din("qT", [D, QS], dt.bfloat16),
        "xT": din("xT", [D, LK], dt.bfloat16),
        "Wq": din("Wq", [D, D], dt.bfloat16),
        "Wk": din("Wk", [D, D], dt.bfloat16),
        "Wv": din("Wv", [D, D], dt.bfloat16),
        "Wo": din("Wo", [D, D], dt.bfloat16),
        # packed per-partition constants: cols 0-7 bq, 8-15 bk, 16-23 kbias,
        # 24-27 q_mask (by query tile)
        "consts": din("consts", [128, 28], dt.float32),
        "out": nc.dram_tensor("out", [QS, D], dt.bfloat16,
                              kind="ExternalOutput").ap(),
    }
    if os.environ.get("KDEBUG"):
        aps["dbg_acc"] = nc.dram_tensor(
            "dbg_acc", [8, 128, 2, QS], dt.float32, kind="ExternalOutput").ap()
        aps["dbg_scb"] = nc.dram_tensor(
            "dbg_scb", [8, 2 * QS], dt.float32, kind="ExternalOutput").ap()
        aps["dbg_ots"] = nc.dram_tensor(
            "dbg_ots", [8, 128, QS], dt.float32, kind="ExternalOutput").ap()

    with tile.TileContext(nc) as tc:
        _body(tc, dt, mybir, aps)

    nc.compile()
    return nc


def _body(tc, dt, mybir, aps):
    from contextlib import ExitStack
    from concourse.tile import add_dep_helper

    ALU = mybir.AluOpType
    AF = mybir.ActivationFunctionType
    nc = tc.nc
    with ExitStack() as ctx:
        const = ctx.enter_context(tc.tile_pool(name="const", bufs=1))
        dbgpool = (ctx.enter_context(tc.tile_pool(name="dbg", bufs=2))
                   if "dbg_acc" in aps else None)
        espool = ctx.enter_context(tc.tile_pool(name="es", bufs=12))
        accpool = ctx.enter_context(tc.tile_pool(name="acc", bufs=3))
        scpool = ctx.enter_context(tc.tile_pool(name="sc", bufs=2))
        spair = ctx.enter_context(tc.tile_pool(name="sp", bufs=2, space="PSUM"))
        oppool = ctx.enter_context(tc.tile_pool(name="op", bufs=2, space="PSUM"))
        gpool = ctx.enter_context(tc.tile_pool(name="g", bufs=2, space="PSUM"))
        opool = ctx.enter_context(tc.tile_pool(name="osb", bufs=3))

        def ctile(shape, dtype, tag):
            return const.tile(shape, dtype, tag=tag, name=tag)

        def gtile():
            return gpool.tile([128, 512], dt.float32, tag="g", name="g")

        # ---- tiny class-0 DMAs ----
        # (bv/bo are folded in EXACTLY on the host: softmax rows sum to 1, so
        # out += q_mask * (bv @ Wo) + bo after the gather)
        cst = ctile([128, 28], dt.float32, "cst")
        nc.sync.dma_start(cst[:], aps["consts"][:, :])

        bq_c = lambda j: cst[:, j:j + 1]
        bk_c = lambda j: cst[:, 8 + j:9 + j]
        kb_c = lambda kt: cst[:, 16 + kt:17 + kt]
        qm_c = lambda qt: cst[:, 24 + qt:25 + qt]

        # ones / junk (vector engine is idle early)
        ones1 = ctile([1, 128], dt.bfloat16, "ones1")
        nc.vector.memset(ones1[:], 1.0)
        ones512 = ctile([1, 512], dt.bfloat16, "ones512")
        nc.vector.memset(ones512[:], 1.0)
        ones64 = ctile([1, 64], dt.bfloat16, "ones64")
        nc.vector.memset(ones64[:], 1.0)
        onescol = ctile([128, 1], dt.bfloat16, "onescol")
        nc.vector.memset(onescol[:], 1.0)
        gjunk = ctile([1, 64], dt.float32, "gjunk")
        ejunk = ctile([1, 16], dt.float32, "ejunk")
        nc.vector.memset(ejunk[:], 1.0)
        # pull the exp ACT table load off the critical path
        nc.scalar.activation(ejunk[:], ejunk[:], AF.Exp, bias=0.0, scale=1.0)

        # ---- keep-alive matmuls: hold the PE HAM clock-gate warm while the
        # input DMAs stream in (PE would otherwise idle >3.4us and re-throttle
        # to 1.2 GHz right as the projections start) ----
        ka = gtile()
        for _ in range(28):
            nc.tensor.matmul(ka[:], ones1[:], ones512[:], start=True, stop=True)

        # ---- input loads, phase-serialized: A: qT+Wq -> B: xT+Wk -> C: Wv
        # -> D: Wo. Without gating all DMA rings run concurrently and share
        # bandwidth, so *everything* (including Wq) lands at ~30us. Big
        # tensors are split 16 ways so each phase spreads over many rings. ----
        engs = [nc.sync, nc.scalar, nc.gpsimd]
        ei = [0]

        def gate(slot, t1, w1, t2, w2):
            # reads 2 corner bytes of every chunk region of both tensors:
            # RAW deps on all their DMAs -> completes when the phase lands
            return nc.vector.tensor_tensor(
                gjunk[0:1, 16 * slot:16 * slot + 16],
                t1[0:1, :, 0:w1 + 1:w1], t2[0:1, :, 0:w2 + 1:w2],
                ALU.add)

        # One InstDMACopy already spreads across all 16 SDMA engines, and
        # transfers >=1MiB run at ~75% of peak; small chunks are descriptor-
        # dominated. So: 1-2 BIG dma_starts per tensor, issued only from
        # sync/scalar (the two HWDGE rings — gpsimd DMAs are slow SWDGE).
        def load_big(name, dram, nfree, nparts):
            tl = ctile([128, 8, nfree], dt.bfloat16, name)
            view = dram.rearrange("(t p) n -> p t n", p=128)
            insts = []
            per = 8 // nparts
            for i in range(nparts):
                ts = slice(per * i, per * (i + 1))
                insts.append(
                    engs[ei[0] % 2].dma_start(tl[:, ts, :], view[:, ts, :]))
                ei[0] += 1
            return tl, insts

        qT_sb, _ = load_big("qT", aps["qT"], QS, 1)
        wq_sb, _ = load_big("wq", aps["Wq"], D, 2)
        # first half of xT rides with class A (kp needs all of xT — letting
        # half stream early shrinks class B so attention starts sooner)
        xT_sb = ctile([128, 8, LK], dt.bfloat16, "xT")
        xview = aps["xT"].rearrange("(t p) n -> p t n", p=128)
        engs[ei[0] % 2].dma_start(xT_sb[:, 0:4, :], xview[:, 0:4, :])
        ei[0] += 1
        gA = gate(0, qT_sb, QS // 2, wq_sb, D // 2)
        xi1 = engs[ei[0] % 2].dma_start(xT_sb[:, 4:8, :], xview[:, 4:8, :])
        ei[0] += 1
        wk_sb, bi2 = load_big("wk", aps["Wk"], D, 2)
        for inst in [xi1] + bi2:
            add_dep_helper(inst.ins, gA.ins, reason="dma class B waits on A")
        gB = gate(1, xT_sb, LK // 2, wk_sb, D // 2)
        wv_sb, ci1 = load_big("wv", aps["Wv"], D, 2)
        for inst in ci1:
            add_dep_helper(inst.ins, gB.ins, reason="dma class C waits on B")
        gC = gate(2, wv_sb, D // 2, wv_sb, D // 2)
        wo_sb, di1 = load_big("wo", aps["Wo"], D, 2)
        for inst in di1:
            add_dep_helper(inst.ins, gC.ins, reason="dma class D waits on C")

        # ---- projections ----
        qTp = [ctile([128, QS], dt.bfloat16, f"qTp{j}") for j in range(8)]

        def q_proj(j):
            ps = gtile()
            for kt in range(8):
                nc.tensor.matmul(ps[:], wq_sb[:, kt, 128 * j:128 * (j + 1)],
                                 qT_sb[:, kt, :], start=(kt == 0), stop=(kt == 7))
            nc.vector.tensor_scalar_add(qTp[j][:], ps[:], bq_c(j))

        kT_sb = [ctile([128, LK], dt.bfloat16, f"kT{j}") for j in range(8)]

        def k_proj_half(j, n):
            c = slice(512 * n, 512 * (n + 1))
            ps = gtile()
            for kt in range(8):
                nc.tensor.matmul(ps[:], wk_sb[:, kt, 128 * j:128 * (j + 1)],
                                 xT_sb[:, kt, c], start=(kt == 0), stop=(kt == 7))
            nc.vector.tensor_scalar_add(kT_sb[j][:, c], ps[:], bk_c(j))

        v_sb = [ctile([128, D], dt.bfloat16, f"v{t}") for t in range(8)]

        def v_proj(t, n):
            c = slice(512 * n, 512 * (n + 1))
            ps = gtile()
            for kd in range(8):
                nc.tensor.matmul(ps[:], xT_sb[:, kd, 128 * t:128 * (t + 1)],
                                 wv_sb[:, kd, c], start=(kd == 0), stop=(kd == 7))
            nc.vector.tensor_copy(v_sb[t][:, c], ps[:])

        # ---- attention ----
        oTs = [ctile([128, QS], dt.bfloat16, f"oTs{j}") for j in range(8)]
        es_tiles = {}
        acc_last = {}
        scb_of = {}

        def s_stage(j, kt):
            kc = slice(128 * kt, 128 * (kt + 1))
            sp = spair.tile([128, 2, QS], dt.float32, tag="sp", name="sp")
            nc.tensor.matmul(sp[:, 0, :], kT_sb[j][0:64, kc],
                             qTp[j][0:64, :], start=True, stop=True)
            nc.tensor.matmul(sp[:, 1, :], kT_sb[j][64:128, kc],
                             qTp[j][64:128, :], start=True, stop=True)
            es = espool.tile([128, 2, QS], dt.bfloat16, tag="es", name="es")
            nc.scalar.activation(es[:], sp[:], AF.Exp,
                                 bias=kb_c(kt), scale=0.125)
            es_tiles[(j, kt)] = es
            if kt == 0:
                acc_last[j] = es
            else:
                # flat 2D APs so the DVE picks the 2x bf16 mode (a 3D
                # [p,1,512] slice forced the 1x fallback: 727ns vs ~420)
                a = accpool.tile([128, 2, QS], dt.bfloat16, tag="acc", name="acc")
                prev = acc_last[j]
                nc.vector.tensor_add(
                    a[:].rearrange("p h q -> p (h q)"),
                    prev[:].rearrange("p h q -> p (h q)"),
                    es[:].rearrange("p h q -> p (h q)"))
                acc_last[j] = a

        def o_stage(j, kt, oP):
            hA, hB = 2 * j, 2 * j + 1
            es = es_tiles.pop((j, kt))
            # skip_group_check: the sim's PSUM zero-region bookkeeping mixes
            # up partition-base offsets (>0) with intra-partition addresses;
            # the two groups live on disjoint partitions so HW per-element
            # has_written handles them fine.
            nc.tensor.matmul(oP[0:64, :], v_sb[kt][:, 64 * hA:64 * hA + 64],
                             es[:, 0, :], start=(kt == 0), stop=(kt == 7),
                             tile_position=(0, 0), skip_group_check=True)
            nc.tensor.matmul(oP[64:128, :], v_sb[kt][:, 64 * hB:64 * hB + 64],
                             es[:, 1, :], start=(kt == 0), stop=(kt == 7),
                             tile_position=(0, 64), skip_group_check=True)

        def den_stage(j):
            a = acc_last.pop(j)
            if "dbg_acc" in aps:
                dbg = dbgpool.tile([128, 2, QS], dt.float32, tag="da", name="da")
                nc.vector.tensor_copy(dbg[:], a[:])
                nc.scalar.dma_start(aps["dbg_acc"][j, :, :, :], dbg[:])
            # reciprocal_approx_fast (custom DVE op) drops the partition base
            # of its input AP — it must read partition 0. So the two
            # denominators go to partition 0 of two separate PSUM tiles.
            dpA = gtile()
            nc.tensor.matmul(dpA[0:1, :], onescol[:], a[:, 0, :],
                             start=True, stop=True)
            dpB = gtile()
            nc.tensor.matmul(dpB[0:1, :], onescol[:], a[:, 1, :],
                             start=True, stop=True)
            sca = scpool.tile([1, 2 * QS], dt.float32, tag="sca", name="sca")
            scb = scpool.tile([1, 2 * QS], dt.bfloat16, tag="scb", name="scb")
            nc.vector.reciprocal_approx_fast(out=sca[:, 0:QS], in_=dpA[0:1, :])
            nc.vector.reciprocal_approx_fast(out=sca[:, QS:2 * QS],
                                             in_=dpB[0:1, :])
            nc.scalar.copy(scb[:], sca[:])  # cast on ScalarE: DVE is loaded
            if "dbg_scb" in aps:
                nc.scalar.dma_start(aps["dbg_scb"][j:j + 1, :], sca[:])
            scb_of[j] = scb

        def sr_stage(j, oP):
            scb = scb_of.pop(j)
            sr = gtile()
            nc.tensor.matmul(sr[0:64, :], ones64[:], scb[:, 0:QS],
                             start=True, stop=True)
            nc.tensor.matmul(sr[64:128, :], ones64[:], scb[:, QS:2 * QS],
                             start=True, stop=True, tile_position=(0, 64),
                             skip_group_check=True)
            # DVE can read at most one PSUM operand: stage sr to SBUF first
            srs = scpool.tile([128, QS], dt.bfloat16, tag="srs", name="srs")
            nc.vector.tensor_copy(srs[:], sr[:])
            nc.vector.tensor_mul(oTs[j][:], oP[:], srs[:])
            if "dbg_ots" in aps:
                dbg = dbgpool.tile([128, QS], dt.float32, tag="do", name="do")
                nc.vector.tensor_copy(dbg[:], oP[:])
                nc.scalar.dma_start(aps["dbg_ots"][j, :, :], dbg[:])

        # out-projection early work: partial j-chains staged to SBUF during
        # pairs 6/7 (transient PSUM use) + one PSUM-resident chain in a freed
        # O-accumulator buffer, so the post-attention drain is short.
        stage_sbuf = {}

        def stage_partial(qt, n, upto):
            c = slice(512 * n, 512 * (n + 1))
            qr = slice(128 * qt, 128 * (qt + 1))
            ps = gtile()
            for jj in range(upto):
                nc.tensor.matmul(ps[:], oTs[jj][:, qr], wo_sb[:, jj, c],
                                 start=(jj == 0), stop=(jj == upto - 1))
            st = ctile([128, 512], dt.float32, f"stg{qt}{n}")
            nc.vector.tensor_scalar_mul(st[:], ps[:], qm_c(qt))  # pre-scale
            stage_sbuf[(qt, n)] = (st, upto)

        fps_state = {}

        def fps_emit(count):
            # qt0/n0 chain in a recycled op-pool buffer (free after sr(6))
            if "ap" not in fps_state:
                fps_state["ap"] = oppool.tile([128, QS], dt.float32,
                                              tag="op", name="op")
                fps_state["next_j"] = 0
            for _ in range(count):
                j = fps_state["next_j"]
                nc.tensor.matmul(fps_state["ap"][:], oTs[j][:, 0:128],
                                 wo_sb[:, j, 0:512], start=(j == 0),
                                 stop=(j == 7))
                fps_state["next_j"] += 1

        dei = [0]

        def out_dma(ot, qt, n):
            # rotate result-tile writes across engines (2KB descriptors)
            qr = slice(128 * qt, 128 * (qt + 1))
            c = slice(512 * n, 512 * (n + 1))
            engs[dei[0] % 2].dma_start(aps["out"][qr, c], ot[:])
            dei[0] += 1

        def out_epilogue(ps_ap, qt, n):
            ot = opool.tile([128, 512], dt.bfloat16, tag="osb", name="osb")
            nc.vector.tensor_scalar_mul(ot[:], ps_ap, qm_c(qt))
            out_dma(ot, qt, n)

        def drain_staged(qt, n):
            st, upto = stage_sbuf[(qt, n)]
            c = slice(512 * n, 512 * (n + 1))
            qr = slice(128 * qt, 128 * (qt + 1))
            ps = gtile()
            for jj in range(upto, 8):
                nc.tensor.matmul(ps[:], oTs[jj][:, qr], wo_sb[:, jj, c],
                                 start=(jj == upto), stop=(jj == 7))
            ot = opool.tile([128, 512], dt.bfloat16, tag="osb", name="osb")
            nc.vector.scalar_tensor_tensor(
                ot[:], ps[:], qm_c(qt), st[:],
                op0=ALU.mult, op1=ALU.add)
            out_dma(ot, qt, n)

        # ---- schedule ----
        for j in range(8):
            q_proj(j)
        k_proj_half(0, 0)
        k_proj_half(0, 1)
        k_proj_half(1, 0)
        k_proj_half(1, 1)

        # per-(pair, kt) PE filler emissions
        fillers = {}
        for kt in range(6):  # V n=0 tiles just-in-time for pair 0's O stages
            fillers[(0, kt)] = [lambda t=kt + 2: v_proj(t, 0)]
        fillers[(0, 6)] = [lambda: k_proj_half(2, 0)]
        fillers[(0, 7)] = [lambda: k_proj_half(2, 1)]
        fillers[(1, 0)] = [lambda: v_proj(0, 1)]
        fillers[(1, 1)] = [lambda: v_proj(1, 1)]
        fillers[(1, 2)] = [lambda: v_proj(2, 1)]
        fillers[(1, 4)] = [lambda: k_proj_half(3, 0)]
        fillers[(1, 5)] = [lambda: k_proj_half(3, 1)]
        fillers[(2, 0)] = [lambda: v_proj(3, 1)]
        fillers[(2, 1)] = [lambda: v_proj(4, 1)]
        fillers[(2, 2)] = [lambda: v_proj(5, 1)]
        fillers[(2, 4)] = [lambda: k_proj_half(4, 0)]
        fillers[(2, 5)] = [lambda: k_proj_half(4, 1)]
        fillers[(3, 0)] = [lambda: v_proj(6, 1)]
        fillers[(3, 1)] = [lambda: v_proj(7, 1)]
        fillers[(3, 4)] = [lambda: k_proj_half(5, 0)]
        fillers[(3, 5)] = [lambda: k_proj_half(5, 1)]
        fillers[(4, 4)] = [lambda: k_proj_half(6, 0)]
        fillers[(4, 5)] = [lambda: k_proj_half(6, 1)]
        fillers[(5, 4)] = [lambda: k_proj_half(7, 0)]
        fillers[(5, 5)] = [lambda: k_proj_half(7, 1)]
        # early out-proj: stage partial j-chains to SBUF as soon as their
        # oTs[j] exist (sr_stage(j) runs at pair j+1 kt2), so the post-
        # attention drain only finishes short chains.
        fillers[(4, 6)] = [lambda: stage_partial(0, 1, 4)]
        fillers[(4, 7)] = [lambda: stage_partial(1, 0, 4)]
        fillers[(5, 6)] = [lambda: stage_partial(1, 1, 5)]
        fillers[(5, 7)] = [lambda: stage_partial(2, 1, 5)]
        fillers[(6, 3)] = [lambda: stage_partial(3, 0, 6)]
        fillers[(6, 5)] = [lambda: stage_partial(3, 1, 6)]
        fillers[(7, 4)] = [lambda: fps_emit(4)]
        fillers[(7, 6)] = [lambda: fps_emit(3)]

        LOOK = 5
        s_cursor = [0]

        def advance_s(upto):
            while s_cursor[0] < min(upto, 64):
                jj, kk = divmod(s_cursor[0], 8)
                s_stage(jj, kk)
                s_cursor[0] += 1

        # blocks of 2 stages: [S,S] [O,O] [fillers] — batching same-shape
        # matmuls halves the PE tiling-mode switch drains.
        # Emit a full pair of S stages BEFORE the first (Wv-gated) v_projs so
        # the exp pipeline starts as soon as xT+Wk land, even if Wv is late.
        advance_s(8)
        v_proj(0, 0)
        v_proj(1, 0)
        oP_prev = None
        for j in range(8):
            oP = oppool.tile([128, QS], dt.float32, tag="op", name="op")
            for kt2 in range(0, 8, 2):
                advance_s(8 * j + kt2 + 2 + LOOK)
                o_stage(j, kt2, oP)
                o_stage(j, kt2 + 1, oP)
                if kt2 == 2 and j > 0:
                    sr_stage(j - 1, oP_prev)
                for f in fillers.get((j, kt2), []):
                    f()
                for f in fillers.get((j, kt2 + 1), []):
                    f()
            den_stage(j)
            oP_prev = oP

        # ---- drain ----
        # den(7) already emitted; cover its recip window with the j<7 matmuls
        # of the one unstaged tile, then scale pair 7 and finish everything.
        ps_qt2n0 = gtile()
        for jj in range(7):
            nc.tensor.matmul(ps_qt2n0[:], oTs[jj][:, 256:384],
                             wo_sb[:, jj, 0:512], start=(jj == 0), stop=False)
        sr_stage(7, oP_prev)
        fps_emit(1)  # j=7 for qt0/n0
        out_epilogue(fps_state["ap"][:], 0, 0)
        nc.tensor.matmul(ps_qt2n0[:], oTs[7][:, 256:384], wo_sb[:, 7, 0:512],
                         start=False, stop=True)
        out_epilogue(ps_qt2n0[:], 2, 0)
        for qt, n in ((3, 0), (3, 1), (2, 1), (1, 1), (0, 1), (1, 0)):
            drain_staged(qt, n)


def get_nc():
    if "nc" not in _NC_CACHE:
        _NC_CACHE["nc"] = _build_nc()
    return _NC_CACHE["nc"]


def make_in_maps(q, x, q_mask, k_mask, Wq, bq, Wk, bk, Wv, bv, Wo, bo):
    """Host-side shard/layout prep. Returns in_maps for cores 0..7."""
    wq_b = Wq.astype(BF16)
    wk_b = Wk.astype(BF16)
    wv_b = Wv.astype(BF16)
    wo_b = Wo.astype(BF16)
    bq_p = bq.astype(np.float32).reshape(8, 128).T
    bk_p = bk.astype(np.float32).reshape(8, 128).T

    in_maps = []
    for c in range(NCORES):
        b, qh = c // 2, c % 2
        qs = slice(QS * qh, QS * (qh + 1))
        kbias = np.where(k_mask[b] != 0, 0.0, NEG).astype(np.float32)
        consts = np.empty((128, 28), np.float32)
        consts[:, 0:8] = bq_p
        consts[:, 8:16] = bk_p
        consts[:, 16:24] = kbias.reshape(8, 128).T
        consts[:, 24:28] = q_mask[b, qs].astype(np.float32).reshape(4, 128).T
        in_maps.append({
            "qT": np.ascontiguousarray(q[b, qs, :].T).astype(BF16),
            "xT": np.ascontiguousarray(x[b].T).astype(BF16),
            "Wq": wq_b, "Wk": wk_b, "Wv": wv_b, "Wo": wo_b,
            "consts": np.ascontiguousarray(consts),
        })
    return in_maps


def kernel(q, x, q_mask, k_mask, Wq, bq, Wk, bk, Wv, bv, Wo, bo):
    from concourse import bass_utils

    q = np.asarray(q, np.float32)
    x = np.asarray(x, np.float32)
    q_mask = np.asarray(q_mask)
    k_mask = np.asarray(k_mask)

    nc = get_nc()
    in_maps = make_in_maps(q, x, q_mask, k_mask, Wq, bq, Wk, bk, Wv, bv, Wo, bo)
    res = bass_utils.run_bass_kernel_spmd(nc, in_maps, core_ids=list(range(NCORES)))

    out = np.empty((B, LQ, D), np.float32)
    for c in range(NCORES):
        b, qh = c // 2, c % 2
        out[b, QS * qh:QS * (qh + 1), :] = res.results[c]["out"]
    # exact host-side bias fold: attn rows sum to 1 pre-q_mask, so
    # out = (attn@V)@Wo  on device  and  +q_mask*(bv@Wo) + bo  here.
    bvwo = np.asarray(bv, np.float32) @ np.asarray(Wo, np.float32)
    bo_f = np.asarray(bo, np.float32)
    if np.any(bvwo) or np.any(bo_f):
        out += (q_mask.astype(np.float32)[:, :, None] * bvwo[None, None, :]
                + bo_f[None, None, :])
    return out



# revision 1
# speedup vs baseline: 1.3005x; 1.3005x over previous
"""Multi-head attention (B=4, L=1024, D=1024, H=16) on 8 TRN2 NeuronCores.

Sharding: pure data-parallel over (batch, query-half) — core c handles batch
c//2, query rows [512*(c%2), 512*(c%2+1)). No collectives; the host
concatenates the 8 output slices.

v2 rewrite (from trace analysis of the v1 kernel):
  * DMA loads are phase-serialized (qT+Wq -> xT+Wk -> Wv+Wo) via gate ops +
    manual deps so the first projection starts at ~9us instead of ~33us
    (v1 let all 16 DMA rings run concurrently -> everything landed at ~30us).
  * Keep-alive matmuls warm the PE HAM clock-gate during the load window.
  * Attention starts right after Q-proj + K-proj(0) (~25us, v1: ~66us);
    remaining K/V projections are PE-queue filler inside the pair stream.
  * O matmuls are column-tiled (two M=64 heads at tile_position (0,0)/(0,64))
    instead of two serial M=65 — the ones-column denominator is replaced by
    a DVE+GPSIMD exp-sum accumulation and two col-tiled M=1 matmuls.
  * Each pair's scale (sr) matmuls are deferred ~2us of PE work into the next
    pair so the reciprocal chain never blocks the in-order PE queue (v1 lost
    ~4.9us/pair to this), and the reciprocal is the single-pass approx_fast.
  * Scale multiply reads O-PSUM and sr-PSUM directly (no oTu copy).
  * bv bias is folded into the V cast (kills 16 K=1 bias matmuls).

Layouts (all transposed, no transposes anywhere):
  Q^T[vd, q] = Wq(lhsT) @ qT(rhs)  (+bq per-partition)
  K^T[vd, k] = Wk(lhsT) @ xT(rhs)  (+bk per-partition)
  V  [k, vd] = xT(lhsT) @ Wv(rhs)  (+bv via DVE add of bv_rep)
  S^T[k, 2, q] = K^T_h(lhsT, K=64) @ Q^T_h  per head PAIR, row-tiled
  es = exp(S^T/8 + kmask_bias)   (ScalarE, PSUM->SBUF bf16)
  acc = sum_kt es                (DVE half + GPSIMD half, bf16)
  den = ones^T @ acc             (two col-tiled M=1 matmuls)
  O^T[128, q] = [V_h0|V_h1](lhsT, M=64 each, col-tiled) @ es
  oTs = O^T * (1/den broadcast)  (DVE, both operands PSUM)
  out[q, d] = (oTs.T @ Wo) * q_mask + bo  (DVE STT epilogue)
"""

import os

os.environ.setdefault("MYCRO_LOCAL_CACHE", "1")

import numpy as np
import ml_dtypes

BF16 = ml_dtypes.bfloat16

B, LQ, LK = 4, 1024, 1024
D = 1024  # QD = KD = VD
H, DH = 16, 64
QS = 512  # queries per core
NCORES = 8
NEG = -1e4  # additive key-mask bias (exp(-1e4) == 0)

_NC_CACHE = {}


def _build_nc():
    import concourse.bacc as bacc
    import concourse.mybir as mybir
    import concourse.tile as tile

    dt = mybir.dt

    nc = bacc.Bacc(
        "TRN2",
        debug=False,
        target_bir_lowering=False,
        num_devices=NCORES,
    )

    def din(name, shape, dtype):
        return nc.dram_tensor(name, shape, dtype, kind="ExternalInput").ap()

    aps = {
        "qT": din("qT", [D, QS], dt.bfloat16),
        "xT": din("xT", [D, LK], dt.bfloat16),
        "Wq": din("Wq", [D, D], dt.bfloat16),
        "Wk": din("Wk", [D, D], dt.bfloat16),
        "Wv": din("Wv", [D, D], dt.bfloat16),
        "Wo": din("Wo", [D, D], dt.bfloat16),
        # packed per-partition constants: cols 0-7 bq, 8-15 bk, 16-23 kbias,
        # 24-27 q_mask (by query tile)
        "consts": din("consts", [128, 28], dt.float32),
        "out": nc.dram_tensor("out", [QS, D], dt.bfloat16,
                              kind="ExternalOutput").ap(),
    }
    if os.environ.get("KDEBUG"):
        aps["dbg_acc"] = nc.dram_tensor(
            "dbg_acc", [8, 128, 2, QS], dt.float32, kind="ExternalOutput").ap()
        aps["dbg_scb"] = nc.dram_tensor(
            "dbg_scb", [8, 2 * QS], dt.float32, kind="ExternalOutput").ap()
        aps["dbg_ots"] = nc.dram_tensor(
            "dbg_ots", [8, 128, QS], dt.float32, kind="ExternalOutput").ap()

    with tile.TileContext(nc) as tc:
        _body(tc, dt, mybir, aps)

    nc.compile()
    return nc


def _body(tc, dt, mybir, aps):
    from contextlib import ExitStack
    from concourse.tile import add_dep_helper

    ALU = mybir.AluOpType
    AF = mybir.ActivationFunctionType
    nc = tc.nc
    with ExitStack() as ctx:
        const = ctx.enter_context(tc.tile_pool(name="const", bufs=1))
        dbgpool = (ctx.enter_context(tc.tile_pool(name="dbg", bufs=2))
                   if "dbg_acc" in aps else None)
        espool = ctx.enter_context(tc.tile_pool(name="es", bufs=12))
        accpool = ctx.enter_context(tc.tile_pool(name="acc", bufs=3))
        scpool = ctx.enter_context(tc.tile_pool(name="sc", bufs=2))
        spair = ctx.enter_context(tc.tile_pool(name="sp", bufs=2, space="PSUM"))
        oppool = ctx.enter_context(tc.tile_pool(name="op", bufs=2, space="PSUM"))
        gpool = ctx.enter_context(tc.tile_pool(name="g", bufs=2, space="PSUM"))
        opool = ctx.enter_context(tc.tile_pool(name="osb", bufs=3))

        def ctile(shape, dtype, tag):
            return const.tile(shape, dtype, tag=tag, name=tag)

        def gtile():
            return gpool.tile([128, 512], dt.float32, tag="g", name="g")

        # ---- tiny class-0 DMAs ----
        # (bv/bo are folded in EXACTLY on the host: softmax rows sum to 1, so
        # out += q_mask * (bv @ Wo) + bo after the gather)
        cst = ctile([128, 28], dt.float32, "cst")
        nc.sync.dma_start(cst[:], aps["consts"][:, :])

        bq_c = lambda j: cst[:, j:j + 1]
        bk_c = lambda j: cst[:, 8 + j:9 + j]
        kb_c = lambda kt: cst[:, 16 + kt:17 + kt]
        qm_c = lambda qt: cst[:, 24 + qt:25 + qt]

        # ones / junk (vector engine is idle early)
        ones1 = ctile([1, 128], dt.bfloat16, "ones1")
        nc.vector.memset(ones1[:], 1.0)
        ones512 = ctile([1, 512], dt.bfloat16, "ones512")
        nc.vector.memset(ones512[:], 1.0)
        ones64 = ctile([1, 64], dt.bfloat16, "ones64")
        nc.vector.memset(ones64[:], 1.0)
        onescol = ctile([128, 1], dt.bfloat16, "onescol")
        nc.vector.memset(onescol[:], 1.0)
        gjunk = ctile([1, 64], dt.float32, "gjunk")
        ejunk = ctile([1, 16], dt.float32, "ejunk")
        nc.vector.memset(ejunk[:], 1.0)
        # pull the exp ACT table load off the critical path
        nc.scalar.activation(ejunk[:], ejunk[:], AF.Exp, bias=0.0, scale=1.0)

        # ---- keep-alive matmuls: hold the PE HAM clock-gate warm while the
        # input DMAs stream in (PE would otherwise idle >3.4us and re-throttle
        # to 1.2 GHz right as the projections start) ----
        ka = gtile()
        for _ in range(28):
            nc.tensor.matmul(ka[:], ones1[:], ones512[:], start=True, stop=True)

        # ---- input loads, phase-serialized: A: qT+Wq -> B: xT+Wk -> C: Wv
        # -> D: Wo. Without gating all DMA rings run concurrently and share
        # bandwidth, so *everything* (including Wq) lands at ~30us. Big
        # tensors are split 16 ways so each phase spreads over many rings. ----
        engs = [nc.sync, nc.scalar, nc.gpsimd]
        ei = [0]

        def gate(slot, t1, w1, t2, w2):
            # reads 2 corner bytes of every chunk region of both tensors:
            # RAW deps on all their DMAs -> completes when the phase lands
            return nc.vector.tensor_tensor(
                gjunk[0:1, 16 * slot:16 * slot + 16],
                t1[0:1, :, 0:w1 + 1:w1], t2[0:1, :, 0:w2 + 1:w2],
                ALU.add)

        # One InstDMACopy already spreads across all 16 SDMA engines, and
        # transfers >=1MiB run at ~75% of peak; small chunks are descriptor-
        # dominated. So: 1-2 BIG dma_starts per tensor, issued only from
        # sync/scalar (the two HWDGE rings — gpsimd DMAs are slow SWDGE).
        def load_big(name, dram, nfree, nparts):
            tl = ctile([128, 8, nfree], dt.bfloat16, name)
            view = dram.rearrange("(t p) n -> p t n", p=128)
            insts = []
            per = 8 // nparts
            for i in range(nparts):
                ts = slice(per * i, per * (i + 1))
                insts.append(
                    engs[ei[0] % 2].dma_start(tl[:, ts, :], view[:, ts, :]))
                ei[0] += 1
            return tl, insts

        qT_sb, _ = load_big("qT", aps["qT"], QS, 1)
        wq_sb, _ = load_big("wq", aps["Wq"], D, 2)
        # first half of xT rides with class A (kp needs all of xT — letting
        # half stream early shrinks class B so attention starts sooner)
        xT_sb = ctile([128, 8, LK], dt.bfloat16, "xT")
        xview = aps["xT"].rearrange("(t p) n -> p t n", p=128)
        engs[ei[0] % 2].dma_start(xT_sb[:, 0:4, :], xview[:, 0:4, :])
        ei[0] += 1
        gA = gate(0, qT_sb, QS // 2, wq_sb, D // 2)
        xi1 = engs[ei[0] % 2].dma_start(xT_sb[:, 4:8, :], xview[:, 4:8, :])
        ei[0] += 1
        wk_sb, bi2 = load_big("wk", aps["Wk"], D, 2)
        for inst in [xi1] + bi2:
            add_dep_helper(inst.ins, gA.ins, reason="dma class B waits on A")
        gB = gate(1, xT_sb, LK // 2, wk_sb, D // 2)
        wv_sb, ci1 = load_big("wv", aps["Wv"], D, 2)
        for inst in ci1:
            add_dep_helper(inst.ins, gB.ins, reason="dma class C waits on B")
        gC = gate(2, wv_sb, D // 2, wv_sb, D // 2)
        wo_sb, di1 = load_big("wo", aps["Wo"], D, 2)
        for inst in di1:
            add_dep_helper(inst.ins, gC.ins, reason="dma class D waits on C")

        # ---- projections ----
        qTp = [ctile([128, QS], dt.bfloat16, f"qTp{j}") for j in range(8)]

        def q_proj(j):
            ps = gtile()
            for kt in range(8):
                nc.tensor.matmul(ps[:], wq_sb[:, kt, 128 * j:128 * (j + 1)],
                                 qT_sb[:, kt, :], start=(kt == 0), stop=(kt == 7))
            nc.vector.tensor_scalar_add(qTp[j][:], ps[:], bq_c(j))

        kT_sb = [ctile([128, LK], dt.bfloat16, f"kT{j}") for j in range(8)]

        def k_proj_half(j, n):
            c = slice(512 * n, 512 * (n + 1))
            ps = gtile()
            for kt in range(8):
                nc.tensor.matmul(ps[:], wk_sb[:, kt, 128 * j:128 * (j + 1)],
                                 xT_sb[:, kt, c], start=(kt == 0), stop=(kt == 7))
            nc.vector.tensor_scalar_add(kT_sb[j][:, c], ps[:], bk_c(j))

        v_sb = [ctile([128, D], dt.bfloat16, f"v{t}") for t in range(8)]

        def v_proj(t, n):
            c = slice(512 * n, 512 * (n + 1))
            ps = gtile()
            for kd in range(8):
                nc.tensor.matmul(ps[:], xT_sb[:, kd, 128 * t:128 * (t + 1)],
                                 wv_sb[:, kd, c], start=(kd == 0), stop=(kd == 7))
            nc.vector.tensor_copy(v_sb[t][:, c], ps[:])

        # ---- attention ----
        oTs = [ctile([128, QS], dt.bfloat16, f"oTs{j}") for j in range(8)]
        es_tiles = {}
        acc_last = {}
        scb_of = {}

        def s_stage(j, kt):
            kc = slice(128 * kt, 128 * (kt + 1))
            sp = spair.tile([128, 2, QS], dt.float32, tag="sp", name="sp")
            nc.tensor.matmul(sp[:, 0, :], kT_sb[j][0:64, kc],
                             qTp[j][0:64, :], start=True, stop=True)
            nc.tensor.matmul(sp[:, 1, :], kT_sb[j][64:128, kc],
                             qTp[j][64:128, :], start=True, stop=True)
            es = espool.tile([128, 2, QS], dt.bfloat16, tag="es", name="es")
            nc.scalar.activation(es[:], sp[:], AF.Exp,
                                 bias=kb_c(kt), scale=0.125)
            es_tiles[(j, kt)] = es
            if kt == 0:
                acc_last[j] = es
            else:
                # flat 2D APs so the DVE picks the 2x bf16 mode (a 3D
                # [p,1,512] slice forced the 1x fallback: 727ns vs ~420)
                a = accpool.tile([128, 2, QS], dt.bfloat16, tag="acc", name="acc")
                prev = acc_last[j]
                nc.vector.tensor_add(
                    a[:].rearrange("p h q -> p (h q)"),
                    prev[:].rearrange("p h q -> p (h q)"),
                    es[:].rearrange("p h q -> p (h q)"))
                acc_last[j] = a

        def o_stage(j, kt, oP):
            hA, hB = 2 * j, 2 * j + 1
            es = es_tiles.pop((j, kt))
            # skip_group_check: the sim's PSUM zero-region bookkeeping mixes
            # up partition-base offsets (>0) with intra-partition addresses;
            # the two groups live on disjoint partitions so HW per-element
            # has_written handles them fine.
            nc.tensor.matmul(oP[0:64, :], v_sb[kt][:, 64 * hA:64 * hA + 64],
                             es[:, 0, :], start=(kt == 0), stop=(kt == 7),
                             tile_position=(0, 0), skip_group_check=True)
            nc.tensor.matmul(oP[64:128, :], v_sb[kt][:, 64 * hB:64 * hB + 64],
                             es[:, 1, :], start=(kt == 0), stop=(kt == 7),
                             tile_position=(0, 64), skip_group_check=True)

        def den_stage(j):
            a = acc_last.pop(j)
            if "dbg_acc" in aps:
                dbg = dbgpool.tile([128, 2, QS], dt.float32, tag="da", name="da")
                nc.vector.tensor_copy(dbg[:], a[:])
                nc.scalar.dma_start(aps["dbg_acc"][j, :, :, :], dbg[:])
            # reciprocal_approx_fast (custom DVE op) drops the partition base
            # of its input AP — it must read partition 0. So the two
            # denominators go to partition 0 of two separate PSUM tiles.
            dpA = gtile()
            nc.tensor.matmul(dpA[0:1, :], onescol[:], a[:, 0, :],
                             start=True, stop=True)
            dpB = gtile()
            nc.tensor.matmul(dpB[0:1, :], onescol[:], a[:, 1, :],
                             start=True, stop=True)
            sca = scpool.tile([1, 2 * QS], dt.float32, tag="sca", name="sca")
            scb = scpool.tile([1, 2 * QS], dt.bfloat16, tag="scb", name="scb")
            nc.vector.reciprocal_approx_fast(out=sca[:, 0:QS], in_=dpA[0:1, :])
            nc.vector.reciprocal_approx_fast(out=sca[:, QS:2 * QS],
                                             in_=dpB[0:1, :])
            nc.scalar.copy(scb[:], sca[:])  # cast on ScalarE: DVE is loaded
            if "dbg_scb" in aps:
                nc.scalar.dma_start(aps["dbg_scb"][j:j + 1, :], sca[:])
            scb_of[j] = scb

        def sr_stage(j, oP):
            scb = scb_of.pop(j)
            sr = gtile()
            nc.tensor.matmul(sr[0:64, :], ones64[:], scb[:, 0:QS],
                             start=True, stop=True)
            nc.tensor.matmul(sr[64:128, :], ones64[:], scb[:, QS:2 * QS],
                             start=True, stop=True, tile_position=(0, 64),
                             skip_group_check=True)
            # DVE can read at most one PSUM operand: stage sr to SBUF first
            srs = scpool.tile([128, QS], dt.bfloat16, tag="srs", name="srs")
            nc.vector.tensor_copy(srs[:], sr[:])
            nc.vector.tensor_mul(oTs[j][:], oP[:], srs[:])
            if "dbg_ots" in aps:
                dbg = dbgpool.tile([128, QS], dt.float32, tag="do", name="do")
                nc.vector.tensor_copy(dbg[:], oP[:])
                nc.scalar.dma_start(aps["dbg_ots"][j, :, :], dbg[:])

        # out-projection early work: partial j-chains staged to SBUF during
        # pairs 6/7 (transient PSUM use) + one PSUM-resident chain in a freed
        # O-accumulator buffer, so the post-attention drain is short.
        stage_sbuf = {}

        def stage_partial(qt, n, upto):
            c = slice(512 * n, 512 * (n + 1))
            qr = slice(128 * qt, 128 * (qt + 1))
            ps = gtile()
            for jj in range(upto):
                nc.tensor.matmul(ps[:], oTs[jj][:, qr], wo_sb[:, jj, c],
                                 start=(jj == 0), stop=(jj == upto - 1))
            st = ctile([128, 512], dt.float32, f"stg{qt}{n}")
            nc.vector.tensor_scalar_mul(st[:], ps[:], qm_c(qt))  # pre-scale
            stage_sbuf[(qt, n)] = (st, upto)

        fps_state = {}

        def fps_emit(count):
            # qt0/n0 chain in a recycled op-pool buffer (free after sr(6))
            if "ap" not in fps_state:
                fps_state["ap"] = oppool.tile([128, QS], dt.float32,
                                              tag="op", name="op")
                fps_state["next_j"] = 0
            for _ in range(count):
                j = fps_state["next_j"]
                nc.tensor.matmul(fps_state["ap"][:], oTs[j][:, 0:128],
                                 wo_sb[:, j, 0:512], start=(j == 0),
                                 stop=(j == 7))
                fps_state["next_j"] += 1

        dei = [0]

        def out_dma(ot, qt, n):
            # rotate result-tile writes across engines (2KB descriptors)
            qr = slice(128 * qt, 128 * (qt + 1))
            c = slice(512 * n, 512 * (n + 1))
            engs[dei[0] % 2].dma_start(aps["out"][qr, c], ot[:])
            dei[0] += 1

        def out_epilogue(ps_ap, qt, n):
            ot = opool.tile([128, 512], dt.bfloat16, tag="osb", name="osb")
            nc.vector.tensor_scalar_mul(ot[:], ps_ap, qm_c(qt))
            out_dma(ot, qt, n)

        def drain_staged(qt, n):
            st, upto = stage_sbuf[(qt, n)]
            c = slice(512 * n, 512 * (n + 1))
            qr = slice(128 * qt, 128 * (qt + 1))
            ps = gtile()
            for jj in range(upto, 8):
                nc.tensor.matmul(ps[:], oTs[jj][:, qr], wo_sb[:, jj, c],
                                 start=(jj == upto), stop=(jj == 7))
            ot = opool.tile([128, 512], dt.bfloat16, tag="osb", name="osb")
            nc.vector.scalar_tensor_tensor(
                ot[:], ps[:], qm_c(qt), st[:],
                op0=ALU.mult, op1=ALU.add)
            out_dma(ot, qt, n)

        # ---- schedule ----
        for j in range(8):
            q_proj(j)
        k_proj_half(0, 0)
        k_proj_half(0, 1)
        k_proj_half(1, 0)
        k_proj_half(1, 1)

        # per-(pair, kt) PE filler emissions
        fillers = {}
        for kt in range(6):  # V n=0 tiles just-in-time for pair 0's O stages
            fillers[(0, kt)] = [lambda t=kt + 2: v_proj(t, 0)]
        fillers[(0, 6)] = [lambda: k_proj_half(2, 0)]
        fillers[(0, 7)] = [lambda: k_proj_half(2, 1)]
        fillers[(1, 0)] = [lambda: v_proj(0, 1)]
        fillers[(1, 1)] = [lambda: v_proj(1, 1)]
        fillers[(1, 2)] = [lambda: v_proj(2, 1)]
        fillers[(1, 4)] = [lambda: k_proj_half(3, 0)]
        fillers[(1, 5)] = [lambda: k_proj_half(3, 1)]
        fillers[(2, 0)] = [lambda: v_proj(3, 1)]
        fillers[(2, 1)] = [lambda: v_proj(4, 1)]
        fillers[(2, 2)] = [lambda: v_proj(5, 1)]
        fillers[(2, 4)] = [lambda: k_proj_half(4, 0)]
        fillers[(2, 5)] = [lambda: k_proj_half(4, 1)]
        fillers[(3, 0)] = [lambda: v_proj(6, 1)]
        fillers[(3, 1)] = [lambda: v_proj(7, 1)]
        fillers[(3, 4)] = [lambda: k_proj_half(5, 0)]
        fillers[(3, 5)] = [lambda: k_proj_half(5, 1)]
        fillers[(4, 4)] = [lambda: k_proj_half(6, 0)]
        fillers[(4, 5)] = [lambda: k_proj_half(6, 1)]
        fillers[(5, 4)] = [lambda: k_proj_half(7, 0)]
        fillers[(5, 5)] = [lambda: k_proj_half(7, 1)]
        # early out-proj: stage partial j-chains to SBUF as soon as their
        # oTs[j] exist (sr_stage(j) runs at pair j+1 kt2), so the post-
        # attention drain only finishes short chains.
        fillers[(4, 6)] = [lambda: stage_partial(0, 1, 4)]
        fillers[(4, 7)] = [lambda: stage_partial(1, 0, 4)]
        fillers[(5, 6)] = [lambda: stage_partial(1, 1, 5)]
        fillers[(5, 7)] = [lambda: stage_partial(2, 1, 5)]
        fillers[(6, 3)] = [lambda: stage_partial(3, 0, 6)]
        fillers[(6, 5)] = [lambda: stage_partial(3, 1, 6)]
        fillers[(7, 4)] = [lambda: fps_emit(4)]
        fillers[(7, 6)] = [lambda: fps_emit(3)]

        LOOK = 5
        s_cursor = [0]

        def advance_s(upto):
            while s_cursor[0] < min(upto, 64):
                jj, kk = divmod(s_cursor[0], 8)
                s_stage(jj, kk)
                s_cursor[0] += 1

        # blocks of 2 stages: [S,S] [O,O] [fillers] — batching same-shape
        # matmuls halves the PE tiling-mode switch drains.
        # Emit a full pair of S stages BEFORE the first (Wv-gated) v_projs so
        # the exp pipeline starts as soon as xT+Wk land, even if Wv is late.
        advance_s(8)
        v_proj(0, 0)
        v_proj(1, 0)
        oP_prev = None
        for j in range(8):
            oP = oppool.tile([128, QS], dt.float32, tag="op", name="op")
            for kt2 in range(0, 8, 2):
                advance_s(8 * j + kt2 + 2 + LOOK)
                o_stage(j, kt2, oP)
                o_stage(j, kt2 + 1, oP)
                if kt2 == 2 and j > 0:
                    sr_stage(j - 1, oP_prev)
                for f in fillers.get((j, kt2), []):
                    f()
                for f in fillers.get((j, kt2 + 1), []):
                    f()
            den_stage(j)
            oP_prev = oP

        # ---- drain ----
        # den(7) already emitted; cover its recip window with the j<7 matmuls
        # of the one unstaged tile, then scale pair 7 and finish everything.
        ps_qt2n0 = gtile()
        for jj in range(7):
            nc.tensor.matmul(ps_qt2n0[:], oTs[jj][:, 256:384],
                             wo_sb[:, jj, 0:512], start=(jj == 0), stop=False)
        sr_stage(7, oP_prev)
        fps_emit(1)  # j=7 for qt0/n0
        out_epilogue(fps_state["ap"][:], 0, 0)
        nc.tensor.matmul(ps_qt2n0[:], oTs[7][:, 256:384], wo_sb[:, 7, 0:512],
                         start=False, stop=True)
        out_epilogue(ps_qt2n0[:], 2, 0)
        for qt, n in ((3, 0), (3, 1), (2, 1), (1, 1), (0, 1), (1, 0)):
            drain_staged(qt, n)


def get_nc():
    if "nc" not in _NC_CACHE:
        _NC_CACHE["nc"] = _build_nc()
    return _NC_CACHE["nc"]


def make_in_maps(q, x, q_mask, k_mask, Wq, bq, Wk, bk, Wv, bv, Wo, bo):
    """Host-side shard/layout prep. Returns in_maps for cores 0..7."""
    wq_b = Wq.astype(BF16)
    wk_b = Wk.astype(BF16)
    wv_b = Wv.astype(BF16)
    wo_b = Wo.astype(BF16)
    bq_p = bq.astype(np.float32).reshape(8, 128).T
    bk_p = bk.astype(np.float32).reshape(8, 128).T

    in_maps = []
    for c in range(NCORES):
        b, qh = c // 2, c % 2
        qs = slice(QS * qh, QS * (qh + 1))
        kbias = np.where(k_mask[b] != 0, 0.0, NEG).astype(np.float32)
        consts = np.empty((128, 28), np.float32)
        consts[:, 0:8] = bq_p
        consts[:, 8:16] = bk_p
        consts[:, 16:24] = kbias.reshape(8, 128).T
        consts[:, 24:28] = q_mask[b, qs].astype(np.float32).reshape(4, 128).T
        in_maps.append({
            "qT": np.ascontiguousarray(q[b, qs, :].T).astype(BF16),
            "xT": np.ascontiguousarray(x[b].T).astype(BF16),
            "Wq": wq_b, "Wk": wk_b, "Wv": wv_b, "Wo": wo_b,
            "consts": np.ascontiguousarray(consts),
        })
    return in_maps


def kernel(q, x, q_mask, k_mask, Wq, bq, Wk, bk, Wv, bv, Wo, bo):
    from concourse import bass_utils

    q = np.asarray(q, np.float32)
    x = np.asarray(x, np.float32)
    q_mask = np.asarray(q_mask)
    k_mask = np.asarray(k_mask)

    nc = get_nc()
    in_maps = make_in_maps(q, x, q_mask, k_mask, Wq, bq, Wk, bk, Wv, bv, Wo, bo)
    res = bass_utils.run_bass_kernel_spmd(nc, in_maps, core_ids=list(range(NCORES)))

    out = np.empty((B, LQ, D), np.float32)
    for c in range(NCORES):
        b, qh = c // 2, c % 2
        out[b, QS * qh:QS * (qh + 1), :] = res.results[c]["out"]
    # exact host-side bias fold: attn rows sum to 1 pre-q_mask, so
    # out = (attn@V)@Wo  on device  and  +q_mask*(bv@Wo) + bo  here.
    bvwo = np.asarray(bv, np.float32) @ np.asarray(Wo, np.float32)
    bo_f = np.asarray(bo, np.float32)
    if np.any(bvwo) or np.any(bo_f):
        out += (q_mask.astype(np.float32)[:, :, None] * bvwo[None, None, :]
                + bo_f[None, None, :])
    return out

